# revision 2
# baseline (speedup 1.0000x reference)
"""Trainium2 Bass kernel for nn_CorrTorch: 27-shift 3D correlation + 1x1x1 conv.

Math (B=1, C=32, D=H=W=64, NOFF=27):
  cv[(k,c), s] = x1[c,s] * pad(x2)[c, s + off_k] / sqrt(C)    (864 x 64^3)
  out[o, s]    = sum_{k,c} conv_w[o, k*32+c] * cv[(k,c), s] + conv_b[o]

Sharding: D axis split across 8 cores (8 planes each), 1-voxel halo baked into
the per-core x2 slab on the host. No collectives.

Per-core device strategy:
  - 3 "replica groups" of 32 channels on partitions 0..95; group g holds data
    pre-shifted by dx=g along W (host-baked), so all 27 shifts reduce to
    9 DVE passes (dz,dy in {0,1,2}^2), each a single contiguous bf16
    tensor_tensor multiply in 2x mode over 96 partitions.
  - The 1x1 conv becomes 9 accumulated matmuls (K=96 contraction chunks) with
    M=27 output channels.  M<32, so 4 spatial subtiles are processed
    concurrently via tile_position column-tiling (col group g4 -> psum
    partitions 32*g4..32*g4+26).
  - ScalarE evicts PSUM -> SBUF fp32 with the conv bias applied per-partition,
    then HWDGE DMA writes straight to HBM.
"""

import numpy as np
import ml_dtypes

import concourse.bass as bass
import concourse.mybir as mybir
import concourse.tile as tile
from concourse.bass_utils import run_bass_kernel_spmd

C = 32
D = 64
H = 64
W = 64
NOFF = 27
NCORES = 8
DLOC = D // NCORES          # 8 output planes per core
NSLAB = DLOC + 2            # 10 padded x2 planes per core
HP = H + 2                  # 66
WP = W + 2                  # 66
PLANE_F = HP * WP           # 4356 elements per padded plane per partition
G = 3                       # dx replica groups
P96 = G * C                 # 96 partitions used by products / contraction
NPASS = 9                   # (dz, dy) passes
ROWS_PER_ROUND = 32         # output rows per psum round
NROUND = H // ROWS_PER_ROUND  # 2 rounds per plane
TN = ROWS_PER_ROUND * W     # 2048 columns per cv tile
NSUB = 4                    # col-tiled spatial subtiles per round
SUB = TN // NSUB            # 512 columns per matmul

BF16 = mybir.dt.bfloat16
F32 = mybir.dt.float32

_wsplit_ctr = [0]


def _split_sync_waits(nc, max_waits=1):
    """Walrus in this container accepts at most one sync wait per instruction.
    Hoist excess waits onto NoOp instructions inserted just before, on the
    same engine (same-engine program order preserves the semantics)."""
    for fn in nc.m.functions:
        for bb in fn.blocks:
            new = []
            changed = False
            for ins in bb.instructions:
                si = ins.sync_info
                if si is not None and len(si.on_wait) > max_waits:
                    waits = list(si.on_wait)
                    excess, keep = waits[:-max_waits], waits[-max_waits:]
                    for i in range(0, len(excess), max_waits):
                        _wsplit_ctr[0] += 1
                        new.append(
                            mybir.InstNoOp(
                                name=f"wsplit-{_wsplit_ctr[0]}",
                                engine=ins.engine,
                                sync_info=mybir.SyncInfo(
                                    on_wait=excess[i : i + max_waits], on_update=[]
                                ),
                            )
                        )
                    ins.sync_info = mybir.SyncInfo(
                        on_wait=keep, on_update=list(si.on_update)
                    )
                    changed = True
                new.append(ins)
            if changed:
                bb.instructions = new


def build_program():
    nc = bass.Bass()

    x1r = nc.dram_tensor("x1r", [DLOC, P96, H * W], BF16, kind="ExternalInput")
    x2r = nc.dram_tensor("x2r", [NSLAB, P96, PLANE_F], BF16, kind="ExternalInput")
    wts = nc.dram_tensor("wts", [P96, NPASS * NOFF], BF16, kind="ExternalInput")
    bias = nc.dram_tensor("bias", [128, 1], F32, kind="ExternalInput")
    out = nc.dram_tensor("out", [NOFF, DLOC * H * W], F32, kind="ExternalOutput")

    with tile.TileContext(nc) as tc:
        with (
            tc.tile_pool(name="wt", bufs=1) as wt_pool,
            tc.tile_pool(name="x2", bufs=5) as x2_pool,
            tc.tile_pool(name="x1", bufs=3) as x1_pool,
            tc.tile_pool(name="cv", bufs=6) as cv_pool,
            tc.tile_pool(name="stage", bufs=3) as stage_pool,
            tc.tile_pool(name="psum", bufs=2, space="PSUM") as psum_pool,
        ):
            wt_tile = wt_pool.tile([P96, NPASS * NOFF], BF16)
            nc.sync.dma_start(out=wt_tile[:], in_=wts[:])
            bias_tile = wt_pool.tile([128, 1], F32)
            nc.sync.dma_start(out=bias_tile[:], in_=bias[:])

            x2t = {}

            def load_x2_plane(p):
                t = x2_pool.tile([P96, HP, WP], BF16, tag="x2plane")
                nc.sync.dma_start(out=t[:], in_=x2r[p])
                x2t[p] = t

            for p in range(3):
                load_x2_plane(p)

            for d in range(DLOC):
                if d + 3 < NSLAB:
                    load_x2_plane(d + 3)
                x1t = x1_pool.tile([P96, H * W], BF16, tag="x1plane")
                nc.sync.dma_start(out=x1t[:], in_=x1r[d])

                for half in range(NROUND):
                    psum = psum_pool.tile([128, SUB], F32)
                    for dz in range(3):
                        for dy in range(3):
                            j = 3 * dz + dy
                            cv = cv_pool.tile([P96, TN], BF16, tag="cv")
                            r0 = half * ROWS_PER_ROUND + dy
                            nc.vector.tensor_mul(
                                out=cv[:],
                                in0=x1t[:, half * TN : (half + 1) * TN],
                                in1=x2t[d + dz][:, r0 : r0 + ROWS_PER_ROUND, 0:W],
                            )
                            for g4 in range(NSUB):
                                nc.tensor.matmul(
                                    psum[32 * g4 : 32 * g4 + NOFF, :],
                                    lhsT=wt_tile[:, j * NOFF : (j + 1) * NOFF],
                                    rhs=cv[:, g4 * SUB : (g4 + 1) * SUB],
                                    start=(j == 0),
                                    stop=(j == NPASS - 1),
                                    tile_position=(0, 32 * g4),
                                )
                    stage = stage_pool.tile([128, SUB], F32, tag="stage")
                    nc.scalar.activation(
                        stage[:],
                        psum[:],
                        mybir.ActivationFunctionType.Identity,
                        bias=bias_tile[:],
                    )
                    base = d * (H * W) + half * TN
                    for g4 in range(NSUB):
                        nc.sync.dma_start(
                            out=out[0:NOFF, base + g4 * SUB : base + (g4 + 1) * SUB],
                            in_=stage[32 * g4 : 32 * g4 + NOFF, :],
                        )

    _split_sync_waits(nc)
    return nc


_PROGRAM = None


def _get_program():
    global _PROGRAM
    if _PROGRAM is None:
        _PROGRAM = build_program()
    return _PROGRAM


def _prep_inputs(in1, in2, conv_w, conv_b):
    """Build the 8 per-core input maps (bf16 layout prep on host)."""
    x1 = np.ascontiguousarray(np.asarray(in1, np.float32).reshape(C, D, H, W))
    x2 = np.ascontiguousarray(np.asarray(in2, np.float32).reshape(C, D, H, W))
    scale = 1.0 / np.sqrt(np.float32(C))
    Wk = (np.asarray(conv_w, np.float32) * scale).reshape(NOFF, NOFF, C)  # [o,k,c]

    wts = np.zeros((P96, NPASS * NOFF), np.float32)
    for dz in range(3):
        for dy in range(3):
            j = 3 * dz + dy
            for g in range(3):
                k = 9 * dz + 3 * dy + g
                wts[32 * g : 32 * g + C, j * NOFF : (j + 1) * NOFF] = Wk[:, k, :].T
    wts = wts.astype(ml_dtypes.bfloat16)

    bias128 = np.zeros((128, 1), np.float32)
    cb = np.asarray(conv_b, np.float32)
    for g4 in range(4):
        bias128[32 * g4 : 32 * g4 + NOFF, 0] = cb

    # Global zero-padded x2: pad plane/row/col index = global index + 1.
    x2p = np.zeros((C, D + 2, HP, WP), np.float32)
    x2p[:, 1 : D + 1, 1 : H + 1, 1 : W + 1] = x2

    in_maps = []
    for m in range(NCORES):
        slab = x2p[:, DLOC * m : DLOC * m + NSLAB]  # [C,10,66,66]
        flat = slab.reshape(C, -1)
        flat = np.concatenate([flat, np.zeros((C, 4), np.float32)], axis=1)
        # replica g = flat shifted by g (dx preshift), cut back to slab planes
        x2rep = np.stack(
            [flat[:, g : g + NSLAB * PLANE_F] for g in range(G)], axis=0
        )  # [3, C, 10*4356]
        x2rep = (
            x2rep.reshape(G * C, NSLAB, PLANE_F)
            .transpose(1, 0, 2)
            .astype(ml_dtypes.bfloat16)
        )  # [10, 96, 4356]

        x1c = x1[:, DLOC * m : DLOC * (m + 1)].reshape(C, -1)  # [C, 8*4096]
        x1rep = (
            np.tile(x1c, (G, 1))
            .reshape(P96, DLOC, H * W)
            .transpose(1, 0, 2)
            .astype(ml_dtypes.bfloat16)
        )  # [8, 96, 4096]

        in_maps.append(
            {
                "x1r": np.ascontiguousarray(x1rep),
                "x2r": np.ascontiguousarray(x2rep),
                "wts": np.ascontiguousarray(wts),
                "bias": bias128,
            }
        )
    return in_maps


def kernel(in1, in2, conv_w, conv_b):
    nc = _get_program()
    in_maps = _prep_inputs(in1, in2, conv_w, conv_b)
    res = run_bass_kernel_spmd(nc, in_maps, core_ids=list(range(NCORES)))
    outs = [r["out"].reshape(NOFF, DLOC, H, W) for r in res.results]
    full = np.concatenate(outs, axis=1)  # [27, 64, 64, 64]
    return full[None].astype(np.float32)  # [1, 27, 64, 64, 64]


# revision 11
# speedup vs baseline: 21939.2951x; 21939.2951x over previous
"""Trainium2 Bass kernel for nn_CorrTorch: 27-shift 3D correlation + 1x1x1 conv.

Math (B=1, C=32, D=H=W=64, NOFF=27):
  cv[(k,c), s] = x1[c,s] * pad(x2)[c, s + off_k] / sqrt(C)    (864 x 64^3)
  out[o, s]    = sum_{k,c} conv_w[o, k*32+c] * cv[(k,c), s] + conv_b[o]

Sharding: D axis split across 8 cores (8 planes each), 1-voxel halo baked into
the per-core x2 slab on the host. No collectives.

Per-core device strategy:
  - 3 "replica groups" of 32 channels on partitions 0..95; group g holds data
    pre-shifted by dx=g along W (host-baked), so all 27 shifts reduce to
    9 DVE passes (dz,dy in {0,1,2}^2), each a single contiguous bf16
    tensor_tensor multiply in 2x mode over 96 partitions.
  - The 1x1 conv becomes 9 accumulated matmuls (K=96 contraction chunks) with
    M=27 output channels.  M<32, so 4 spatial subtiles are processed
    concurrently via tile_position column-tiling (col group g4 -> psum
    partitions 32*g4..32*g4+26).
  - ScalarE evicts PSUM -> SBUF fp32 with the conv bias applied per-partition,
    then HWDGE DMA writes straight to HBM.
"""

import numpy as np
import ml_dtypes

import concourse.bass as bass
import concourse.mybir as mybir
import concourse.tile as tile
from concourse.bass_utils import run_bass_kernel_spmd

C = 32
D = 64
H = 64
W = 64
NOFF = 27
NCORES = 8
DLOC = D // NCORES          # 8 output planes per core
NSLAB = DLOC + 2            # 10 padded x2 planes per core
HP = H + 2                  # 66
WP = W + 2                  # 66
PLANE_F = HP * WP           # 4356 elements per padded plane per partition
G = 3                       # dx replica groups
P96 = G * C                 # 96 partitions used by products / contraction
NPASS = 9                   # (dz, dy) passes
TN = H * W                  # 4096 columns per cv tile (one full plane)
SUB = 512                   # columns per matmul (one PSUM bank)
NSUBT = TN // SUB           # 8 spatial subtiles per plane
NSUB = 4                    # col-tiled concurrent matmul groups

BF16 = mybir.dt.bfloat16
F32 = mybir.dt.float32

_wsplit_ctr = [0]


def _split_sync_waits(nc, max_waits=1):
    """Walrus in this container accepts at most one sync wait per instruction.
    Hoist excess waits onto NoOp instructions inserted just before, on the
    same engine (same-engine program order preserves the semantics)."""
    for fn in nc.m.functions:
        for bb in fn.blocks:
            new = []
            changed = False
            for ins in bb.instructions:
                si = ins.sync_info
                if si is not None and len(si.on_wait) > max_waits:
                    waits = list(si.on_wait)
                    excess, keep = waits[:-max_waits], waits[-max_waits:]
                    for i in range(0, len(excess), max_waits):
                        _wsplit_ctr[0] += 1
                        new.append(
                            mybir.InstNoOp(
                                name=f"wsplit-{_wsplit_ctr[0]}",
                                engine=ins.engine,
                                sync_info=mybir.SyncInfo(
                                    on_wait=excess[i : i + max_waits], on_update=[]
                                ),
                            )
                        )
                    ins.sync_info = mybir.SyncInfo(
                        on_wait=keep, on_update=list(si.on_update)
                    )
                    changed = True
                new.append(ins)
            if changed:
                bb.instructions = new


def build_program():
    nc = bass.Bass()

    x1r = nc.dram_tensor("x1r", [DLOC, P96, H * W], BF16, kind="ExternalInput")
    x2r = nc.dram_tensor("x2r", [NSLAB, P96, PLANE_F], BF16, kind="ExternalInput")
    wts = nc.dram_tensor("wts", [P96, NPASS * NOFF], BF16, kind="ExternalInput")
    bias = nc.dram_tensor("bias", [128, 1], F32, kind="ExternalInput")
    out = nc.dram_tensor("out", [NOFF, DLOC * H * W], F32, kind="ExternalOutput")

    with tile.TileContext(nc) as tc:
        with (
            tc.tile_pool(name="wt", bufs=1) as wt_pool,
            tc.tile_pool(name="x2", bufs=5) as x2_pool,
            tc.tile_pool(name="x1", bufs=3) as x1_pool,
            tc.tile_pool(name="cv", bufs=11) as cv_pool,
            tc.tile_pool(name="stage", bufs=3) as stage_pool,
            tc.tile_pool(name="psum", bufs=4, space="PSUM") as psum_pool,
        ):
            wt_tile = wt_pool.tile([P96, NPASS * NOFF], BF16)
            nc.sync.dma_start(out=wt_tile[:], in_=wts[:])
            bias_tile = wt_pool.tile([128, 1], F32)
            nc.sync.dma_start(out=bias_tile[:], in_=bias[:])

            x2t = {}

            def load_x2_plane(p):
                t = x2_pool.tile([P96, HP, WP], BF16, tag="x2plane")
                nc.sync.dma_start(out=t[:], in_=x2r[p])
                x2t[p] = t

            # first compute pass needs only x1[0] + x2[0]; issue those first
            x1t0 = x1_pool.tile([P96, H * W], BF16, tag="x1plane")
            nc.sync.dma_start(out=x1t0[:], in_=x1r[0])
            for p in range(3):
                load_x2_plane(p)

            for d in range(DLOC):
                if d + 3 < NSLAB:
                    load_x2_plane(d + 3)
                if d == 0:
                    x1t = x1t0
                else:
                    x1t = x1_pool.tile([P96, H * W], BF16, tag="x1plane")
                    nc.sync.dma_start(out=x1t[:], in_=x1r[d])

                psums = []
                for _ph in range(2):
                    ps = psum_pool.tile([128, SUB], F32, tag="ps")
                    psums.append(ps)
                for dz in range(3):
                    for dy in range(3):
                        j = 3 * dz + dy
                        cv = cv_pool.tile([P96, TN], BF16, tag="cv")
                        # ~2 passes per plane run on GPSIMD in parallel with
                        # the rest on the (bottleneck) vector engine
                        offload = d < 7 and (dz, dy) in ((1, 1), (2, 1))
                        eng = nc.gpsimd if offload else nc.vector
                        eng.tensor_mul(
                            out=cv[:],
                            in0=x1t[:],
                            in1=x2t[d + dz][:, dy : dy + H, 0:W],
                        )
                        for s in range(NSUBT):
                            nc.tensor.matmul(
                                psums[s // NSUB][32 * (s % NSUB) : 32 * (s % NSUB) + NOFF, :],
                                lhsT=wt_tile[:, j * NOFF : (j + 1) * NOFF],
                                rhs=cv[:, s * SUB : (s + 1) * SUB],
                                start=(j == 0),
                                stop=(j == NPASS - 1),
                                tile_position=(0, 32 * (s % NSUB)),
                            )
                for half in range(2):
                    stage = stage_pool.tile([128, SUB], F32, tag="stage")
                    nc.scalar.activation(
                        stage[:],
                        psums[half][:],
                        mybir.ActivationFunctionType.Identity,
                        bias=bias_tile[:],
                    )
                    base = d * (H * W) + half * (TN // 2)
                    for g4 in range(NSUB):
                        nc.sync.dma_start(
                            out=out[0:NOFF, base + g4 * SUB : base + (g4 + 1) * SUB],
                            in_=stage[32 * g4 : 32 * g4 + NOFF, :],
                        )

    _split_sync_waits(nc)
    return nc


_PROGRAM = None


def _get_program():
    global _PROGRAM
    if _PROGRAM is None:
        _PROGRAM = build_program()
    return _PROGRAM


def _prep_inputs(in1, in2, conv_w, conv_b):
    """Build the 8 per-core input maps (bf16 layout prep on host)."""
    x1 = np.ascontiguousarray(np.asarray(in1, np.float32).reshape(C, D, H, W))
    x2 = np.ascontiguousarray(np.asarray(in2, np.float32).reshape(C, D, H, W))
    scale = 1.0 / np.sqrt(np.float32(C))
    Wk = (np.asarray(conv_w, np.float32) * scale).reshape(NOFF, NOFF, C)  # [o,k,c]

    wts = np.zeros((P96, NPASS * NOFF), np.float32)
    for dz in range(3):
        for dy in range(3):
            j = 3 * dz + dy
            for g in range(3):
                k = 9 * dz + 3 * dy + g
                wts[32 * g : 32 * g + C, j * NOFF : (j + 1) * NOFF] = Wk[:, k, :].T
    wts = wts.astype(ml_dtypes.bfloat16)

    bias128 = np.zeros((128, 1), np.float32)
    cb = np.asarray(conv_b, np.float32)
    for g4 in range(4):
        bias128[32 * g4 : 32 * g4 + NOFF, 0] = cb

    # Global zero-padded x2: pad plane/row/col index = global index + 1.
    x2p = np.zeros((C, D + 2, HP, WP), np.float32)
    x2p[:, 1 : D + 1, 1 : H + 1, 1 : W + 1] = x2

    in_maps = []
    for m in range(NCORES):
        slab = x2p[:, DLOC * m : DLOC * m + NSLAB]  # [C,10,66,66]
        flat = slab.reshape(C, -1)
        flat = np.concatenate([flat, np.zeros((C, 4), np.float32)], axis=1)
        # replica g = flat shifted by g (dx preshift), cut back to slab planes
        x2rep = np.stack(
            [flat[:, g : g + NSLAB * PLANE_F] for g in range(G)], axis=0
        )  # [3, C, 10*4356]
        x2rep = (
            x2rep.reshape(G * C, NSLAB, PLANE_F)
            .transpose(1, 0, 2)
            .astype(ml_dtypes.bfloat16)
        )  # [10, 96, 4356]

        x1c = x1[:, DLOC * m : DLOC * (m + 1)].reshape(C, -1)  # [C, 8*4096]
        x1rep = (
            np.tile(x1c, (G, 1))
            .reshape(P96, DLOC, H * W)
            .transpose(1, 0, 2)
            .astype(ml_dtypes.bfloat16)
        )  # [8, 96, 4096]

        in_maps.append(
            {
                "x1r": np.ascontiguousarray(x1rep),
                "x2r": np.ascontiguousarray(x2rep),
                "wts": np.ascontiguousarray(wts),
                "bias": bias128,
            }
        )
    return in_maps


def kernel(in1, in2, conv_w, conv_b):
    nc = _get_program()
    in_maps = _prep_inputs(in1, in2, conv_w, conv_b)
    res = run_bass_kernel_spmd(nc, in_maps, core_ids=list(range(NCORES)))
    outs = [r["out"].reshape(NOFF, DLOC, H, W) for r in res.results]
    full = np.concatenate(outs, axis=1)  # [27, 64, 64, 64]
    return full[None].astype(np.float32)  # [1, 27, 64, 64, 64]


# revision 14
# speedup vs baseline: 22186.0025x; 1.0112x over previous
"""Trainium2 Bass kernel for nn_CorrTorch: 27-shift 3D correlation + 1x1x1 conv.

Math (B=1, C=32, D=H=W=64, NOFF=27):
  cv[(k,c), s] = x1[c,s] * pad(x2)[c, s + off_k] / sqrt(C)    (864 x 64^3)
  out[o, s]    = sum_{k,c} conv_w[o, k*32+c] * cv[(k,c), s] + conv_b[o]

Sharding: D axis split across 8 cores (8 planes each), 1-voxel halo baked into
the per-core x2 slab on the host. No collectives.

Per-core device strategy:
  - 3 "replica groups" of 32 channels on partitions 0..95; group g holds data
    pre-shifted by dx=g along W (host-baked), so all 27 shifts reduce to
    9 DVE passes (dz,dy in {0,1,2}^2), each a single contiguous bf16
    tensor_tensor multiply in 2x mode over 96 partitions.
  - The 1x1 conv becomes 9 accumulated matmuls (K=96 contraction chunks) with
    M=27 output channels.  M<32, so 4 spatial subtiles are processed
    concurrently via tile_position column-tiling (col group g4 -> psum
    partitions 32*g4..32*g4+26).
  - ScalarE evicts PSUM -> SBUF fp32 with the conv bias applied per-partition,
    then HWDGE DMA writes straight to HBM.
"""

import numpy as np
import ml_dtypes

import concourse.bass as bass
import concourse.mybir as mybir
import concourse.tile as tile
from concourse.bass_utils import run_bass_kernel_spmd

C = 32
D = 64
H = 64
W = 64
NOFF = 27
NCORES = 8
DLOC = D // NCORES          # 8 output planes per core
NSLAB = DLOC + 2            # 10 padded x2 planes per core
HP = H + 2                  # 66
WP = W + 2                  # 66
PLANE_F = HP * WP           # 4356 elements per padded plane per partition
G = 3                       # dx replica groups
P96 = G * C                 # 96 partitions used by products / contraction
NPASS = 9                   # (dz, dy) passes
TN = H * W                  # 4096 columns per cv tile (one full plane)
SUB = 512                   # columns per matmul (one PSUM bank)
NSUBT = TN // SUB           # 8 spatial subtiles per plane
NSUB = 4                    # col-tiled concurrent matmul groups

BF16 = mybir.dt.bfloat16
F32 = mybir.dt.float32

_wsplit_ctr = [0]


def _split_sync_waits(nc, max_waits=1):
    """Walrus in this container accepts at most one sync wait per instruction.
    Hoist excess waits onto NoOp instructions inserted just before, on the
    same engine (same-engine program order preserves the semantics)."""
    for fn in nc.m.functions:
        for bb in fn.blocks:
            new = []
            changed = False
            for ins in bb.instructions:
                si = ins.sync_info
                if si is not None and len(si.on_wait) > max_waits:
                    waits = list(si.on_wait)
                    excess, keep = waits[:-max_waits], waits[-max_waits:]
                    for i in range(0, len(excess), max_waits):
                        _wsplit_ctr[0] += 1
                        new.append(
                            mybir.InstNoOp(
                                name=f"wsplit-{_wsplit_ctr[0]}",
                                engine=ins.engine,
                                sync_info=mybir.SyncInfo(
                                    on_wait=excess[i : i + max_waits], on_update=[]
                                ),
                            )
                        )
                    ins.sync_info = mybir.SyncInfo(
                        on_wait=keep, on_update=list(si.on_update)
                    )
                    changed = True
                new.append(ins)
            if changed:
                bb.instructions = new


def build_program():
    nc = bass.Bass()

    x1r = nc.dram_tensor("x1r", [DLOC, P96, H * W], BF16, kind="ExternalInput")
    x2r = nc.dram_tensor("x2r", [NSLAB, P96, PLANE_F], BF16, kind="ExternalInput")
    wts = nc.dram_tensor("wts", [P96, NPASS * NOFF], BF16, kind="ExternalInput")
    bias = nc.dram_tensor("bias", [128, 1], F32, kind="ExternalInput")
    out = nc.dram_tensor("out", [NOFF, DLOC * H * W], F32, kind="ExternalOutput")

    with tile.TileContext(nc) as tc:
        with (
            tc.tile_pool(name="wt", bufs=1) as wt_pool,
            tc.tile_pool(name="x2", bufs=5) as x2_pool,
            tc.tile_pool(name="x1", bufs=3) as x1_pool,
            tc.tile_pool(name="cv", bufs=11) as cv_pool,
            tc.tile_pool(name="stage", bufs=3) as stage_pool,
            tc.tile_pool(name="psum", bufs=4, space="PSUM") as psum_pool,
        ):
            wt_tile = wt_pool.tile([P96, NPASS * NOFF], BF16)
            nc.sync.dma_start(out=wt_tile[:], in_=wts[:])
            bias_tile = wt_pool.tile([128, 1], F32)
            nc.sync.dma_start(out=bias_tile[:], in_=bias[:])

            x2t = {}

            def load_x2_plane(p):
                t = x2_pool.tile([P96, HP, WP], BF16, tag="x2plane")
                nc.sync.dma_start(out=t[:], in_=x2r[p])
                x2t[p] = t

            # first compute pass needs only x1[0] + x2[0]; issue those first
            x1t0 = x1_pool.tile([P96, H * W], BF16, tag="x1plane")
            nc.sync.dma_start(out=x1t0[:], in_=x1r[0])
            for p in range(3):
                load_x2_plane(p)

            for d in range(DLOC):
                if d + 3 < NSLAB:
                    load_x2_plane(d + 3)
                if d == 0:
                    x1t = x1t0
                else:
                    x1t = x1_pool.tile([P96, H * W], BF16, tag="x1plane")
                    nc.sync.dma_start(out=x1t[:], in_=x1r[d])

                psums = []
                for _ph in range(2):
                    ps = psum_pool.tile([128, SUB], F32, tag="ps")
                    psums.append(ps)
                for dz in range(3):
                    for dy in range(3):
                        j = 3 * dz + dy
                        cv = cv_pool.tile([P96, TN], BF16, tag="cv")
                        # ~2 passes per plane run on GPSIMD in parallel with
                        # the rest on the (bottleneck) vector engine
                        offload = (d < 7 and (dz, dy) in ((1, 1), (2, 2))) or (
                            d == 7 and (dz, dy) == (2, 2)
                        )
                        eng = nc.gpsimd if offload else nc.vector
                        eng.tensor_mul(
                            out=cv[:],
                            in0=x1t[:],
                            in1=x2t[d + dz][:, dy : dy + H, 0:W],
                        )
                        for s in range(NSUBT):
                            nc.tensor.matmul(
                                psums[s // NSUB][32 * (s % NSUB) : 32 * (s % NSUB) + NOFF, :],
                                lhsT=wt_tile[:, j * NOFF : (j + 1) * NOFF],
                                rhs=cv[:, s * SUB : (s + 1) * SUB],
                                start=(j == 0),
                                stop=(j == NPASS - 1),
                                tile_position=(0, 32 * (s % NSUB)),
                            )
                for half in range(2):
                    stage = stage_pool.tile([128, SUB], F32, tag="stage")
                    nc.scalar.activation(
                        stage[:],
                        psums[half][:],
                        mybir.ActivationFunctionType.Identity,
                        bias=bias_tile[:],
                    )
                    base = d * (H * W) + half * (TN // 2)
                    for g4 in range(NSUB):
                        nc.sync.dma_start(
                            out=out[0:NOFF, base + g4 * SUB : base + (g4 + 1) * SUB],
                            in_=stage[32 * g4 : 32 * g4 + NOFF, :],
                        )

    _split_sync_waits(nc)
    return nc


_PROGRAM = None


def _get_program():
    global _PROGRAM
    if _PROGRAM is None:
        _PROGRAM = build_program()
    return _PROGRAM


def _prep_inputs(in1, in2, conv_w, conv_b):
    """Build the 8 per-core input maps (bf16 layout prep on host)."""
    x1 = np.ascontiguousarray(np.asarray(in1, np.float32).reshape(C, D, H, W))
    x2 = np.ascontiguousarray(np.asarray(in2, np.float32).reshape(C, D, H, W))
    scale = 1.0 / np.sqrt(np.float32(C))
    Wk = (np.asarray(conv_w, np.float32) * scale).reshape(NOFF, NOFF, C)  # [o,k,c]

    wts = np.zeros((P96, NPASS * NOFF), np.float32)
    for dz in range(3):
        for dy in range(3):
            j = 3 * dz + dy
            for g in range(3):
                k = 9 * dz + 3 * dy + g
                wts[32 * g : 32 * g + C, j * NOFF : (j + 1) * NOFF] = Wk[:, k, :].T
    wts = wts.astype(ml_dtypes.bfloat16)

    bias128 = np.zeros((128, 1), np.float32)
    cb = np.asarray(conv_b, np.float32)
    for g4 in range(4):
        bias128[32 * g4 : 32 * g4 + NOFF, 0] = cb

    # Global zero-padded x2: pad plane/row/col index = global index + 1.
    x2p = np.zeros((C, D + 2, HP, WP), np.float32)
    x2p[:, 1 : D + 1, 1 : H + 1, 1 : W + 1] = x2

    in_maps = []
    for m in range(NCORES):
        slab = x2p[:, DLOC * m : DLOC * m + NSLAB]  # [C,10,66,66]
        flat = slab.reshape(C, -1)
        flat = np.concatenate([flat, np.zeros((C, 4), np.float32)], axis=1)
        # replica g = flat shifted by g (dx preshift), cut back to slab planes
        x2rep = np.stack(
            [flat[:, g : g + NSLAB * PLANE_F] for g in range(G)], axis=0
        )  # [3, C, 10*4356]
        x2rep = (
            x2rep.reshape(G * C, NSLAB, PLANE_F)
            .transpose(1, 0, 2)
            .astype(ml_dtypes.bfloat16)
        )  # [10, 96, 4356]

        x1c = x1[:, DLOC * m : DLOC * (m + 1)].reshape(C, -1)  # [C, 8*4096]
        x1rep = (
            np.tile(x1c, (G, 1))
            .reshape(P96, DLOC, H * W)
            .transpose(1, 0, 2)
            .astype(ml_dtypes.bfloat16)
        )  # [8, 96, 4096]

        in_maps.append(
            {
                "x1r": np.ascontiguousarray(x1rep),
                "x2r": np.ascontiguousarray(x2rep),
                "wts": np.ascontiguousarray(wts),
                "bias": bias128,
            }
        )
    return in_maps


def kernel(in1, in2, conv_w, conv_b):
    nc = _get_program()
    in_maps = _prep_inputs(in1, in2, conv_w, conv_b)
    res = run_bass_kernel_spmd(nc, in_maps, core_ids=list(range(NCORES)))
    outs = [r["out"].reshape(NOFF, DLOC, H, W) for r in res.results]
    full = np.concatenate(outs, axis=1)  # [27, 64, 64, 64]
    return full[None].astype(np.float32)  # [1, 27, 64, 64, 64]
